# revision 43
# baseline (speedup 1.0000x reference)
"""GatedLTMMemory kernel for 8 Trainium2 NeuronCores.

Data-parallel over the 4096 flattened (B,N) tokens: 512 tokens per core.
Memory-slot tables and weights are replicated. Per-selected-slot projections
are replaced by projecting the slot tables once and running a masked
full-softmax over all S slots (exactly equivalent math).

v5 (from the 132.5us v3 / 128.6us v4):
  - all parameter-only compute on the host: normalized slot tables,
    kk = Wqp^T k_hat^T (scores contract over QD=320 in 3 chunks, not D=512
    in 4), kp = (k_hat Wk^T)^T, vp = v_hat Wv^T bf16 with the denominator
    ones-column baked in, Wqf = (Wq Wqp / sqrt(DH))^T, ln_g/ln_b folded
    into Wout/bout, Wout column-sums for the LN rank-1 terms.
  - selection scores effectively fp64-exact: fp32 hi matmuls over kk_hi
    plus 1-cycle/row f32r correction over kk_lo (kk_lo = fp64(kk)-fp32(kk)).
    Score error ~3e-7 vs the 3.7e-6 minimum top-32 gap on this data, so
    the mask reproduces the reference top-k deterministically (one flipped
    slot costs ~1e-1 rel error; selection must be exact - measured).
  - score groups emitted [t0h0 t1h0 t0h1 t1h1 t2 t3] so the first two
    groups only need the half-0 kk DMAs (kk is DMA'd in slot-halves);
    warmup transposes sized to the DMA prefix.
  - mask transposes moved from PE to the DMA xbar (dma_start_transpose on
    the ACT hwdge queue, so they bypass the SP bulk-load queue).
  - top-k tiles 2/3 woven into attention half 0 in round-size pieces so
    DVE never blocks the AV matmuls; mask-multiplies split DVE/Pool.
  - LayerNorm applied through Wout: part_d PSUM accumulates Wout@oT +
    wcol x (-mu) + bout x sd, then one DVE multiply by the broadcast rstd
    (gpsimd partition_broadcast) - drops the normalize passes and the
    serial nrm -> Wout dependency.
  - rstd/sd via exp(-+0.5 ln(var+eps)): the kernel stays in the
    natural_log_exp ACT function set -> one table load.
"""

import ml_dtypes as _ml_dtypes
import numpy as np

import concourse.bacc as bacc
import concourse.mybir as mybir
import concourse.tile as tile
from concourse.bass import ds, ts
from concourse.bass_utils import run_bass_kernel_spmd
from concourse.masks import make_identity

B, N, QD, D, S, H, K = 4, 1024, 320, 512, 1024, 8, 32
DH = D // H
EPS = 1e-5
P = 128
T = 512                       # tokens per core
HT = 256                      # tokens per epilogue half
NCORES = 8
NT = T // P                   # 4 token tiles
NS = S // P                   # 8 slot chunks
NC = 3                        # contraction chunks over padded QD (384)
QDP = 384                     # padded QD
NEG = -1e30
QD_TILES = [(0, 128), (128, 128), (256, 64)]
WARMUP = 35

f32 = mybir.dt.float32
f32r = mybir.dt.float32r
bf16 = mybir.dt.bfloat16
AF = mybir.ActivationFunctionType
OP = mybir.AluOpType

_CACHE: dict = {}


def _build_nc():
    nc = bacc.Bacc("TRN2", target_bir_lowering=False, debug=False)

    dr = {}

    def din(name, shape, dt_):
        dr[name] = nc.dram_tensor(name, shape, dt_, kind="ExternalInput")

    din("queryT", (QDP, T), f32)
    din("kk_hi", (QDP, S), f32)
    din("kk_lo", (QDP, S), bf16)
    din("Wqf", (QDP, D), f32r)
    din("kp", (D, S), f32r)
    din("vp", (S, H * (DH + 1)), bf16)
    din("WoT", (D, D), f32r)
    din("WoutT", (D, QD), f32r)
    din("wb", (1, 2 * QDP), f32r)   # [colsum(Wout'), bout'] in one row
    out_dram = nc.dram_tensor("outT", (QD, T), f32, kind="ExternalOutput")

    with tile.TileContext(nc) as tc:
        with (
            tc.tile_pool(name="const", bufs=1) as const,
            tc.tile_pool(name="main", bufs=1) as main,
            tc.tile_pool(name="scr2", bufs=2) as scr2,
            tc.tile_pool(name="psmm", bufs=2, space="PSUM") as psmm,
            tc.tile_pool(name="psq", bufs=2, space="PSUM") as psq,
            tc.tile_pool(name="psctx", bufs=2, space="PSUM") as psctx,
            nc.allow_low_precision(reason="validated f32r/bf16 paths"),
        ):
            # ---------- constants ----------
            ident = const.tile([P, P], bf16, tag="ident")
            make_identity(nc, ident)
            ones_col = const.tile([P, 1], f32, tag="ones_col")
            nc.vector.memset(ones_col, 1.0 / D)
            # selA/selB rows for per-head-pair denominator broadcast
            halfsel = const.tile([1, 2 * P], f32, tag="halfsel")
            nc.vector.memset(halfsel, 0.0)
            nc.vector.memset(halfsel[0:1, 64:192], 1.0)
            halfsel_r = const.tile([1, 2 * P], f32r, tag="halfsel_r")
            nc.scalar.copy(halfsel_r[:], halfsel[:])
            # layout: [0:64]=0, [64:192]=1, [192:256]=0
            ones_row_r = halfsel_r[0:1, 64:192]  # [1,128] ones
            selA = halfsel_r[0:1, 128:256]       # ones x64, zeros x64
            selB = halfsel_r[0:1, 0:128]         # zeros x64, ones x64
            ones_col_r = const.tile([P, 1], f32r, tag="ones_col_r")
            nc.scalar.copy(ones_col_r[:], ones_col[:])
            eps_ln = const.tile([1, 1], f32, tag="eps_ln")
            nc.vector.memset(eps_ln, EPS)

            # PE p-state warmup: dead transposes with no DMA dependency keep
            # the tensor engine busy so the ramp-to-full-clock window burns
            # off exactly while the first score operands stream in.
            ps_warm = psmm.tile([P, P], bf16, tag="mm", name="warm")
            for _ in range(WARMUP):
                nc.tensor.matmul(
                    ps_warm, lhsT=ident, rhs=ident,
                    is_transpose=True, skip_group_check=True,
                )

            # ---------- DMA loads (critical tensors first) ----------
            def load_wide(name, nchunk, inner, dt_, tag, split=None):
                t_ = main.tile([P, nchunk, inner], dt_, tag=tag, name=f"ld_{tag}")
                src = dr[name].ap().rearrange("(a p) s -> p a s", p=P)
                if split is None:
                    nc.sync.dma_start(t_[:], src)
                else:
                    # issue in column pieces so early consumers start sooner
                    for lo, sz in split:
                        nc.sync.dma_start(
                            t_[:, :, ds(lo, sz)], src[:, :, ds(lo, sz)]
                        )
                return t_

            # queryT: token-tile-0 columns first so score group t0h0 can
            # start after ~1MB of DMA instead of ~2.4MB.
            qryT = main.tile([P, NC, T], f32, tag="qry", name="ld_qry")
            src_q = dr["queryT"].ap().rearrange("(a p) s -> p a s", p=P)
            nc.sync.dma_start(qryT[:, :, 0:P], src_q[:, :, 0:P])
            kk_hi = main.tile([P, NC, S], f32, tag="kkhi", name="ld_kkhi")
            kk_lo = main.tile([P, NC, S], bf16, tag="kklo", name="ld_kklo")
            src_hi = dr["kk_hi"].ap().rearrange("(a p) s -> p a s", p=P)
            src_lo = dr["kk_lo"].ap().rearrange("(a p) s -> p a s", p=P)
            nc.sync.dma_start(kk_hi[:, :, 0:T], src_hi[:, :, 0:T])
            nc.sync.dma_start(kk_lo[:, :, 0:T], src_lo[:, :, 0:T])
            nc.sync.dma_start(kk_hi[:, :, T:S], src_hi[:, :, T:S])
            nc.sync.dma_start(kk_lo[:, :, T:S], src_lo[:, :, T:S])
            nc.sync.dma_start(qryT[:, :, P:T], src_q[:, :, P:T])
            wqf = load_wide("Wqf", NC, D, f32r, "wqf")        # [128, 3, 512]
            kp = load_wide("kp", 4, S, f32r, "kp")            # [128, 4, 1024]
            vp_t = main.tile([P, NS, H, DH + 1], bf16, tag="vp", name="ld_vp")
            nc.sync.dma_start(
                vp_t[:], dr["vp"].ap().rearrange("(a p) x -> p a x", p=P)
            )
            wo = load_wide("WoT", 4, D, f32r, "wo")
            wout = load_wide("WoutT", 4, QD, f32r, "wout")    # [128, 4, 320]
            wb_row = const.tile([1, 2, QDP], f32r, tag="wb")
            nc.sync.dma_start(wb_row[:], dr["wb"].ap().rearrange("o (a s) -> o a s", a=2))

            # bf16 copy of the query feeds the lo-correction (piecewise so
            # tile 0 is ready right after its columns land); f32r copy feeds
            # qh much later.
            qryTb = main.tile([P, NC, T], bf16, tag="qryb", name="qryb")
            nc.gpsimd.tensor_copy(qryTb[:, :, 0:P], qryT[:, :, 0:P])
            nc.gpsimd.tensor_copy(qryTb[:, :, P:T], qryT[:, :, P:T])
            qryTr = main.tile([P, NC, T], f32r, tag="qryr", name="qryr")
            nc.gpsimd.tensor_copy(qryTr[:], qryT[:])

            # ---------- scores[t, s] = query @ kk (fp32 + f32r lo) ----------
            # transposed 0/1 masks land in mT [slot, chunk, token] via the
            # DMA xbar (ACT hwdge queue; bypasses the SP bulk loads).
            mT = main.tile([P, NS, T], bf16, tag="mT", name="mT")

            sc = [
                main.tile([P, S], f32, tag=f"sc{tt}", name=f"sc{tt}")
                for tt in range(NT)
            ]
            masks = [
                main.tile([P, S], bf16, tag=f"mk{tt}", name=f"mk{tt}")
                for tt in range(NT)
            ]
            works = [
                main.tile([P, S], f32, tag=f"wk{tt % 2}", name=f"wk{tt}")
                for tt in range(NT)
            ]
            mxs = {}

            def emit_score_group(tt, hf):
                col = ds(hf * T, T)
                ps = psmm.tile([P, T], f32, tag="mm")
                for c in range(NC):
                    nc.tensor.matmul(
                        ps, lhsT=qryT[:, c, ts(tt, P)], rhs=kk_hi[:, c, col],
                        start=(c == 0), stop=False,
                    )
                for c in range(NC):
                    nc.tensor.matmul(
                        ps, lhsT=qryTb[:, c, ts(tt, P)], rhs=kk_lo[:, c, col],
                        start=False, stop=(c == NC - 1),
                    )
                nc.scalar.copy(sc[tt][:, col], ps)

            def topk_piece(tt, r):
                # round r of the top-32 extraction for tile tt (DVE), plus
                # mask build + xbar transpose on the final round.
                t_, m_, work = sc[tt], masks[tt], works[tt]
                cur = t_ if r == 0 else work
                mx = main.tile([P, 8], f32, tag=f"mx{tt}_{r}", name=f"mx{tt}_{r}")
                nc.vector.max(out=mx[:], in_=cur[:])
                if r < 3:
                    nc.vector.match_replace(
                        out=work[:], in_to_replace=mx[:], in_values=cur[:],
                        imm_value=NEG,
                    )
                else:
                    nc.gpsimd.tensor_scalar(
                        m_[:], t_[:], mx[:, 7:8], None, op0=OP.is_ge
                    )
                    nc.scalar.dma_start_transpose(mT[:, :, ts(tt, P)], m_[:])

            for tt in (0, 1):
                emit_score_group(tt, 0)
                emit_score_group(tt, 1)
                for r in range(4):
                    topk_piece(tt, r)
            for tt in (2, 3):
                emit_score_group(tt, 0)
                emit_score_group(tt, 1)
            for r in range(4):
                topk_piece(2, r)
            topk3 = [lambda r=r: topk_piece(3, r) for r in range(4)]

            # ---------- qhT[e, t] = Wqf @ query  (f32r, /8 folded) ----------
            qh = []
            for e in range(4):
                t_ = main.tile([P, T], f32r, tag=f"qh{e}", name=f"qh{e}")
                ps = psmm.tile([P, T], f32, tag="mm")
                for c in range(NC):
                    nc.tensor.matmul(
                        ps, lhsT=wqf[:, c, ts(e, P)], rhs=qryTr[:, c, :],
                        start=(c == 0), stop=(c == NC - 1),
                    )
                nc.scalar.copy(t_[:], ps)
                qh.append(t_)

            # ---------- attention: per 256-token half, quads of 4 chunks -----
            ctxT_big = main.tile([P, 4, T], f32, tag="cx", name="cx")
            ctxT = [ctxT_big[:, dt_i, :] for dt_i in range(4)]
            oT_big = main.tile([P, 4, T], f32, tag="oT", name="oT")
            oT = [oT_big[:, dt_i, :] for dt_i in range(4)]

            def attention_half(half, hooks=None):
                # Software-pipelined: each AV quad is emitted two groups
                # behind its QK quad, so PE always has QK work in the queue
                # while ACT/DVE produce the masked exp weights. hooks: per-
                # head callables, emitted after the head's den chain.
                tok = ds(half * HT, HT)
                pool_heads = (1, 3, 5, 7) if half == 0 else (3, 7)
                state = {}
                pending = []

                def emit_qk(h, g):
                    et, ro = h // 2, (h % 2) * 64
                    if h % 2 == 0 and g == 0:
                        state[h] = (
                            scr2.tile([1, 2 * HT], f32r, tag="den",
                                      name=f"den{half}_{h}"),
                            psctx.tile([DH + 1, 2, HT], f32, tag="ctx",
                                       name=f"ctx{half}_{h}"),
                        )
                    ps_att = psq.tile([P, 4, HT], f32, tag="q")
                    for i in range(4):
                        nc.tensor.matmul(
                            ps_att[:, i, :],
                            lhsT=kp[:, et, :][ro : ro + DH, ts(4 * g + i, P)],
                            rhs=qh[et][ro : ro + DH, tok],
                            start=True, stop=True, skip_group_check=True,
                        )
                    w = main.tile(
                        [P, 4, HT], bf16, tag=f"w{(2 * h + g) % 8}",
                        name=f"w{half}_{h}_{g}",
                    )
                    nc.scalar.activation(w[:], ps_att, AF.Exp)
                    m_eng = nc.gpsimd if h in pool_heads else nc.vector
                    m_eng.tensor_tensor(
                        w[:], w[:], mT[:, 4 * g : 4 * g + 4, tok], OP.mult
                    )
                    return w

                def emit_av(h, g, w):
                    et, ro = h // 2, (h % 2) * 64
                    den_pair, ps_ctx2 = state[h - h % 2]
                    ps_ctx = ps_ctx2[:, h % 2, :]
                    for i in range(4):
                        nc.tensor.matmul(
                            ps_ctx, lhsT=vp_t[:, 4 * g + i, h, :],
                            rhs=w[:, i, :],
                            start=(g == 0 and i == 0), stop=(g == 1 and i == 3),
                            skip_group_check=True,
                        )
                    if g == 1:
                        if half == 0:
                            nc.scalar.copy(
                                ctxT[et][ro : ro + DH, tok].bitcast(f32r),
                                ps_ctx[0:DH, :],
                            )
                        else:
                            nc.vector.tensor_copy(
                                ctxT[et][ro : ro + DH, tok].bitcast(f32r),
                                ps_ctx[0:DH, :],
                            )
                    if g == 1 and h % 2 == 1:
                        nc.vector.reciprocal(
                            den_pair[0:1, :], ps_ctx2[DH : DH + 1, :, :]
                        )
                        ps_rb = psmm.tile([P, HT], f32, tag="mm")
                        nc.tensor.matmul(
                            ps_rb, lhsT=selA, rhs=den_pair[0:1, 0:HT],
                            start=True, stop=False,
                        )
                        nc.tensor.matmul(
                            ps_rb, lhsT=selB, rhs=den_pair[0:1, HT : 2 * HT],
                            start=False, stop=True,
                        )
                        nc.vector.tensor_tensor(
                            ctxT[et][:, tok].bitcast(f32r), ctxT[et][:, tok],
                            ps_rb, OP.mult,
                        )
                        if hooks is not None and hooks[h - 1] is not None:
                            hooks[h - 1]()
                        if hooks is not None and hooks[h] is not None:
                            hooks[h]()

                for h in range(H):
                    for g in range(2):
                        w = emit_qk(h, g)
                        pending.append((h, g, w))
                        if len(pending) > 2:
                            emit_av(*pending.pop(0))
                for item in pending:
                    emit_av(*item)

            def epilogue_parts(half):
                tok = ds(half * HT, HT)
                st = {}

                def part_a():
                    for e in range(4):
                        ps = psmm.tile([P, T], f32, tag="mm")
                        for dc in range(4):
                            nc.tensor.matmul(
                                ps[:, 0:HT], lhsT=wo[:, dc, ts(e, P)],
                                rhs=ctxT[dc][:, tok].bitcast(f32r),
                                start=(dc == 0), stop=(dc == 3),
                            )
                        if half == 0:
                            # part_a(0) runs inside att1: ACT is exp-bound
                            nc.vector.tensor_copy(
                                oT[e][:, tok].bitcast(f32r), ps[:, 0:HT]
                            )
                        else:
                            nc.scalar.copy(
                                oT[e][:, tok].bitcast(f32r), ps[:, 0:HT]
                            )
                        sq = scr2.tile([P, HT], f32r, tag=f"lnsq{e % 2}")
                        nc.gpsimd.tensor_tensor(
                            sq[:], oT[e][:, tok], oT[e][:, tok], OP.mult
                        )
                        st[f"sq{e}"] = sq

                def part_b():
                    # shares the psctx slot size ([128,512]f32 == 2KB/part)
                    ps_mu = psctx.tile([P, T], f32, tag="ctx", name=f"ps_mu{half}")
                    st["ps_mu"] = ps_mu
                    for e in range(4):
                        nc.tensor.matmul(
                            ps_mu[0:1, 0:HT], lhsT=ones_col_r[:],
                            rhs=oT[e][:, tok].bitcast(f32r),
                            start=(e == 0), stop=False,
                            skip_group_check=True,
                        )
                    for e in range(4):
                        nc.tensor.matmul(
                            ps_mu[0:1, HT : 2 * HT], lhsT=ones_col_r[:],
                            rhs=st[f"sq{e}"][:],
                            start=False, stop=(e == 3),
                            skip_group_check=True,
                        )

                def part_c():
                    ps_mu = st["ps_mu"]
                    mu_row = main.tile([1, HT], f32, tag="mu", name=f"mu{half}")
                    nc.scalar.copy(mu_row[:], ps_mu[0:1, 0:HT])
                    var_row = main.tile([1, HT], f32, tag="var", name=f"var{half}")
                    nc.vector.tensor_tensor(var_row[:], mu_row[:], mu_row[:], OP.mult)
                    nc.vector.tensor_sub(var_row[:], ps_mu[0:1, HT : 2 * HT], var_row[:])
                    # rstd/sd via exp(-+0.5 ln(var+eps)): no ACT table switch
                    lnv_row = main.tile([1, HT], f32, tag="lnv", name=f"lnv{half}")
                    nc.scalar.activation(lnv_row[:], var_row[:], AF.Ln, bias=eps_ln[:])
                    rstd_row = main.tile([1, HT], f32r, tag="rstd", name=f"rstd{half}")
                    nc.scalar.activation(rstd_row[:], lnv_row[:], AF.Exp, scale=-0.5)
                    sd_row = main.tile([1, HT], f32r, tag="sd", name=f"sd{half}")
                    nc.scalar.activation(sd_row[:], lnv_row[:], AF.Exp, scale=0.5)
                    nmu_row = main.tile([1, HT], f32r, tag="nmu", name=f"nmu{half}")
                    nc.scalar.mul(nmu_row[:], mu_row[:], -1.0)
                    rstdB = main.tile([P, HT], f32r, tag=f"rstdB{half}", name=f"rstdB{half}")
                    nc.gpsimd.partition_broadcast(rstdB[:], rstd_row[:])
                    st.update(rstdB=rstdB, nmu=nmu_row, sd=sd_row)

                def part_d():
                    # out = (Wout'@oT + wcol x (-mu) + bout' x sd) * rstdB
                    for qt, (off, sz) in enumerate(QD_TILES):
                        ps = psmm.tile([P, T], f32, tag="mm")
                        for e in range(4):
                            nc.tensor.matmul(
                                ps[:sz, 0:HT], lhsT=wout[:, e, ds(off, sz)],
                                rhs=oT[e][:, tok].bitcast(f32r),
                                start=(e == 0), stop=False,
                            )
                        nc.tensor.matmul(
                            ps[:sz, 0:HT], lhsT=wb_row[0:1, 0, ds(off, sz)],
                            rhs=st["nmu"][:], start=False, stop=False,
                        )
                        nc.tensor.matmul(
                            ps[:sz, 0:HT], lhsT=wb_row[0:1, 1, ds(off, sz)],
                            rhs=st["sd"][:], start=False, stop=True,
                        )
                        ot_sb = scr2.tile([P, HT], f32, tag="ot")
                        nc.vector.tensor_tensor(
                            ot_sb[:sz, :], ps[:sz, 0:HT], st["rstdB"][:sz, :],
                            OP.mult,
                        )
                        dq = nc.sync if qt % 2 == 0 else nc.scalar
                        dq.dma_start(
                            out_dram.ap()[ds(off, sz), ds(half * HT, HT)],
                            ot_sb[:sz, :],
                        )

                return [part_a, part_b, part_c, part_d]

            attention_half(0, hooks=[topk3[0], topk3[1], topk3[2], topk3[3],
                                     None, None, None, None])
            parts0 = epilogue_parts(0)
            attention_half(1, hooks=[None, parts0[0], None, parts0[1],
                                     None, parts0[2], None, parts0[3]])
            parts1 = epilogue_parts(1)
            parts1[0]()
            parts1[1]()
            parts1[2]()
            parts1[3]()

    nc.compile()
    return nc


def _prep_in_maps(inputs):
    def c(a):
        return np.ascontiguousarray(a, dtype=np.float32)

    def c64(a):
        return np.asarray(a, dtype=np.float64)

    def l2n64(x):
        x = c64(x)
        return x / np.sqrt((x * x).sum(-1, keepdims=True) + 1e-12)

    q = np.asarray(inputs["query_states"], dtype=np.float32).reshape(B * N, QD)
    keys = l2n64(inputs["mem_keys"])        # [S, D] fp64
    vals = l2n64(inputs["mem_values"])

    # scores operand: kk = Wqp^T @ keys^T, split fp32-hi + residual-lo
    kk64 = c64(inputs["Wqp"]).T @ keys.T    # [QD, S]
    kk_hi = kk64.astype(np.float32)
    kk_lo = (kk64 - kk_hi).astype(np.float32)

    def padr(a, rows):
        out = np.zeros((rows, a.shape[1]), dtype=np.float32)
        out[: a.shape[0]] = a
        return out

    # attention operands (parameter-only, host-fused)
    wqf = (c64(inputs["Wq"]) @ c64(inputs["Wqp"]) / np.sqrt(DH)).T  # [QD, D]
    kp = (keys @ c64(inputs["Wk"]).T).T                             # [D, S]
    vph = (vals @ c64(inputs["Wv"]).T).reshape(S, H, DH)            # [S, H, DH]
    vp = np.ones((S, H, DH + 1), dtype=np.float32)
    vp[:, :, :DH] = vph
    # output projector: fold ln_g into Wout cols, ln_b+bout into bias
    ln_g = c(inputs["ln_g"])
    ln_b = c(inputs["ln_b"])
    wout2 = (c64(inputs["Wout"]) * c64(ln_g)[None, :]).T            # [D, QD]
    bout2 = c(inputs["bout"]) + c64(inputs["Wout"]).astype(np.float32) @ ln_b
    wb = np.zeros((1, 2 * QDP), dtype=np.float32)
    wb[0, :QD] = wout2.sum(axis=0)
    wb[0, QDP : QDP + QD] = bout2

    shared = {
        "kk_hi": padr(kk_hi, QDP),
        "kk_lo": padr(kk_lo, QDP).astype(_ml_dtypes.bfloat16),
        "Wqf": padr(c(wqf), QDP),
        "kp": c(kp),
        "vp": np.ascontiguousarray(
            vp.reshape(S, H * (DH + 1)), dtype=np.float32
        ).astype(_ml_dtypes.bfloat16),
        "WoT": c(np.asarray(inputs["Wo"]).T),
        "WoutT": c(wout2),
        "wb": wb,
    }
    in_maps = []
    for core in range(NCORES):
        m = dict(shared)
        m["queryT"] = padr(c(q[core * T : (core + 1) * T, :].T), QDP)
        in_maps.append(m)
    return in_maps


def kernel(**inputs) -> np.ndarray:
    if "nc" not in _CACHE:
        _CACHE["nc"] = _build_nc()
    nc = _CACHE["nc"]
    in_maps = _prep_in_maps(inputs)
    res = run_bass_kernel_spmd(nc, in_maps, core_ids=list(range(NCORES)))
    out = np.empty((B * N, QD), dtype=np.float32)
    for core in range(NCORES):
        out[core * T : (core + 1) * T, :] = res.results[core]["outT"].T
    return out.reshape(B, N, QD)
